# revision 1
# baseline (speedup 1.0000x reference)
"""Trainium2 Bass kernel for the quadtree-sum (CNNDST) problem.

Math: the reference quadtree recursion computes, for each sample b,
    out[b, j] = sum over (r, c) with (r AND c) == j of M[b, 0, r, c]
(j, r, c are 10-bit).  The per-bit AND factorizes, so the ten bit-pairs
(r_k, c_k) can be contracted in any order; contracting pair k maps
A[r_k=0,c_k=0] + A[0,1] + A[1,0] -> B[j_k=0] and A[1,1] -> B[j_k=1].

Layout per core (8 samples, data parallel over 8 cores; 4 groups of 2
samples per core).  Group tile [128 part x 16384 f32]:
    partition p = 64*si + (r0 c0 r1 c1 r2 c2)       (si = sample in pair)
    free f (MSB->LSB) = (r9 c9 r8 c8 ... r3 c3)     (host-side relayout)
Seven in-partition levels contract (r9,c9) .. (r3,c3); with the pair in
the two MSBs of the active array every level is 2 contiguous/2-dim-AP
adds (dst = q00+q01; dst += q10) plus a ScalarE copy of q11, ping-pong
compacting 16384 -> 128 elems/partition.  Design points:
  - in-place two-op accumulation: no tmp buffer, no tmp WAR stalls
  - all adds on DVE; copies on ScalarE; all access patterns contiguous
    or 2-dim with long runs (measured: per-level DVE+Pool splits and a
    3:1 group-level split are both slower - cross-engine sync and Pool's
    ~2x-slower ALUs eat the parallelism gain)
  - level 0 writes no g1 copy: level 1's j9=1 block reads x's q11
    quarter in place
  - the whole add chain runs in bf16: group loads are gpsimd (qPool)
    DMAs that cast f32 DRAM -> bf16 SBUF in flight, so every DVE level
    gets the 2x perf mode (f32 TensorTensor is capped at 1 elem/lane/
    cycle; 2-byte dtypes run 2x); measured global rel err ~1.0e-2 vs
    the 2e-2 gate on the fixed harness input
  - each 8MB group load is 4 f-chunks so level-0 compute starts after
    chunk 1; bulk loads ride qPool (full rate ~390 GB/s, same as qSP),
    w/output DMAs ride qSP
Cross-partition levels (pairs 0..2) are one tiny PE matmul per group
with constant 0/1 W[128,16]: S[8*si+m][e] = sum_p W[p][8si+m] D[p][e],
where m = j&7, e = j>>3.  PSUM -> SBUF staging (ScalarE), one 32KB
output DMA at kernel end; host does the final [16,512] -> [8,1024]
index unscramble (free).
"""

import numpy as np

import concourse.bass as bass
import concourse.tile as tile
from concourse import bacc
from concourse import mybir
import concourse.bass_utils as bass_utils
from concourse.ap import AP

F32 = mybir.dt.float32
BF16 = mybir.dt.bfloat16

SPC = 8
NCORES = 8
XPITCH = 16384
Q = 4096


def _mkap(tile_ap, p_off, p_cnt, off, dims):
    pitch = tile_ap.ap[0][0]
    return AP(
        tile_ap.tensor,
        tile_ap.offset + p_off * pitch + off,
        [[pitch, p_cnt]] + [[s, c] for (s, c) in dims],
    )


def _acc3(eng, dst, a, b, c):
    eng.tensor_add(dst, a, b)
    eng.tensor_add(dst, dst, c)


def _emit_levels(nc, xa, xb, bufB, bufA2, bufBb, d, aeng):
    # all adds of one group ride a single engine (aeng): groups are split
    # 3:1 DVE:Pool at the group level, so no per-level cross-engine syncs
    for f0, fn, eng in ((0, 4096, aeng),):
        _acc3(eng,
              _mkap(bufB, 0, 128, f0, [(1, fn)]),
              _mkap(xa, 0, 128, f0, [(1, fn)]),
              _mkap(xa, 0, 128, Q + f0, [(1, fn)]),
              _mkap(xb, 0, 128, f0, [(1, fn)]))

    # ---- level 1: blocks b0 (src bufB) and b1 (src xb q11, in place) ----
    R = 1024
    for blk, (src, base) in enumerate(((bufB, 0), (xb, Q))):
        for f0, fn, eng in ((0, 1024, aeng),):
            _acc3(eng,
                  _mkap(bufA2, 0, 128, 2 * R * blk + f0, [(1, fn)]),
                  _mkap(src, 0, 128, base + f0, [(1, fn)]),
                  _mkap(src, 0, 128, base + R + f0, [(1, fn)]),
                  _mkap(src, 0, 128, base + 2 * R + f0, [(1, fn)]))
        nc.scalar.copy(_mkap(bufA2, 0, 128, 2 * R * blk + R, [(1, R)]),
                       _mkap(src, 0, 128, base + 3 * R, [(1, R)]))

    # ---- levels 2..6: ping-pong compaction, adds on DVE ----
    cur = bufA2           # bf16 from here on: 2x DVE mode, gate is 2e-2
    for i in range(2, 7):
        R = 1 << (12 - 2 * i)
        P = 1 << i
        nxt = d if i == 6 else (bufBb if i % 2 == 0 else bufA2)
        for b0, nb, eng in ((0, P, aeng),):
            sdims = [(4 * R, nb), (1, R)] if R > 1 else [(4, nb)]
            ddims = [(2 * R, nb), (1, R)] if R > 1 else [(2, nb)]
            _acc3(eng,
                  _mkap(nxt, 0, 128, b0 * 2 * R, ddims),
                  _mkap(cur, 0, 128, b0 * 4 * R, sdims),
                  _mkap(cur, 0, 128, b0 * 4 * R + R, sdims),
                  _mkap(cur, 0, 128, b0 * 4 * R + 2 * R, sdims))
        cdims = [(4 * R, P), (1, R)] if R > 1 else [(4, P)]
        odims = [(2 * R, P), (1, R)] if R > 1 else [(2, P)]
        nc.scalar.copy(_mkap(nxt, 0, 128, R, odims),
                       _mkap(cur, 0, 128, 3 * R, cdims))
        cur = nxt


def _emit_body(nc, m, w_t, oall, pools):
    xapool, xbpool, bpool, dpool, ppool = pools
    for g in range(4):
        xa = xapool.tile([128, 2 * Q], BF16)
        xb = xbpool.tile([128, 2 * Q], BF16)
        for c, xt in ((0, xa), (1, xa), (2, xb), (3, xb)):
            src = AP(m.ap().tensor, g * 128 * XPITCH + c * Q,
                     [[XPITCH, 128], [1, Q]])
            dst = AP(xt.tensor, xt.offset + (c % 2) * Q,
                     [[xt.ap[0][0], 128], [1, Q]])
            # gpsimd (qPool) DMA casts f32 DRAM -> bf16 SBUF in flight:
            # every DVE level then runs in the 2x perf mode
            nc.gpsimd.dma_start(dst, src)

        aeng = nc.vector
        sfx = "v"
        bufB = bpool.tile([128, 4096], BF16, tag="bufB" + sfx)
        bufA2 = bpool.tile([128, 4096], BF16, tag="bufA2" + sfx)
        bufBb = bpool.tile([128, 2048], BF16, tag="bufBb" + sfx)
        d = dpool.tile([128, 128], BF16)
        _emit_levels(nc, xa, xb, bufB, bufA2, bufBb, d, aeng)

        ps = ppool.tile([16, 128], F32)
        nc.tensor.matmul(ps[:], w_t[:], d[:], start=True, stop=True)
        nc.scalar.copy(_mkap(oall, 0, 16, g * 128, [(1, 128)]), ps[:])


def make_w() -> np.ndarray:
    import ml_dtypes
    w = np.zeros((128, 16), ml_dtypes.bfloat16)
    for p in range(128):
        si = p >> 6
        r0, c0 = (p >> 5) & 1, (p >> 4) & 1
        r1, c1 = (p >> 3) & 1, (p >> 2) & 1
        r2, c2 = (p >> 1) & 1, p & 1
        mm = (r0 & c0) | ((r1 & c1) << 1) | ((r2 & c2) << 2)
        w[p][8 * si + mm] = 1.0
    return w


def build(iters: int = 1, timing_internal: bool = False) -> bass.Bass:
    """timing_internal: 'm' is Internal DRAM (no host upload), zero-filled
    on device before the loop - for loop-differencing timing only."""
    nc = bacc.Bacc("TRN2", target_bir_lowering=False, debug=False)
    mkind = "Internal" if timing_internal else "ExternalInput"
    m = nc.dram_tensor("m", [4, 128 * XPITCH], F32, kind=mkind)
    w = nc.dram_tensor("w", [128, 16], BF16, kind="ExternalInput")
    out = nc.dram_tensor("out", [16, 512], F32, kind="ExternalOutput")
    from contextlib import ExitStack

    with tile.TileContext(nc) as tc:
        with ExitStack() as ctx:
            xapool = ctx.enter_context(tc.tile_pool(name="xa", bufs=2))
            xbpool = ctx.enter_context(tc.tile_pool(name="xb", bufs=2))
            bpool = ctx.enter_context(tc.tile_pool(name="b", bufs=1))
            dpool = ctx.enter_context(tc.tile_pool(name="d", bufs=2))
            ppool = ctx.enter_context(tc.tile_pool(name="ps", bufs=2, space="PSUM"))
            wpool = ctx.enter_context(tc.tile_pool(name="w", bufs=1))
            w_t = wpool.tile([128, 16], BF16)
            oall = wpool.tile([16, 512], F32, tag="oall")
            pools = (xapool, xbpool, bpool, dpool, ppool)
            nc.sync.dma_start(w_t[:], w.ap())
            if timing_internal:
                zpool = ctx.enter_context(tc.tile_pool(name="z", bufs=1))
                z = zpool.tile([128, 2048], F32)
                nc.vector.memset(z[:], 0.0)
                for ch in range(32):
                    nc.sync.dma_start(
                        AP(m.ap().tensor,
                           (ch // 8) * 128 * XPITCH + (ch % 8) * 2048,
                           [[XPITCH, 128], [1, 2048]]),
                        AP(z.tensor, z.offset, [[z.ap[0][0], 128], [1, 2048]]))
            if iters == 1:
                _emit_body(nc, m, w_t, oall, pools)
            else:
                with tc.For_i(0, iters, 1):
                    _emit_body(nc, m, w_t, oall, pools)
            nc.sync.dma_start(out.ap(), oall[:])
    nc.compile()
    return nc


def _relayout(M: np.ndarray) -> np.ndarray:
    B = M.shape[0]
    Mv = M.reshape((B,) + (2,) * 20)
    r = {k: 1 + (9 - k) for k in range(10)}
    c = {k: 11 + (9 - k) for k in range(10)}
    perm = [0,
            r[0], c[0], r[1], c[1], r[2], c[2],
            r[9], c[9], r[8], c[8], r[7], c[7], r[6], c[6],
            r[5], c[5], r[4], c[4], r[3], c[3]]
    return np.ascontiguousarray(Mv.transpose(perm)).reshape(B, 64, XPITCH)


def _unscramble(res_cores: list[np.ndarray]) -> np.ndarray:
    outs = []
    for rk in res_cores:
        v = rk.reshape(2, 8, 4, 128)          # [si][m][g][e]
        v = v.transpose(2, 0, 3, 1)           # [g][si][e][m]
        outs.append(v.reshape(8, 1024))
    return np.concatenate(outs, axis=0)


def kernel(**inputs) -> np.ndarray:
    M = np.asarray(inputs["M"], dtype=np.float32)
    B = M.shape[0]
    X = _relayout(M.reshape(B, 1024, 1024))
    X = X.reshape(B // 2, 2 * 64 * XPITCH)
    nc = build(1)
    w = make_w()
    in_maps = [
        {"m": X[4 * k:4 * k + 4].reshape(4, 128 * XPITCH), "w": w}
        for k in range(NCORES)
    ]
    res = bass_utils.run_bass_kernel_spmd(nc, in_maps, core_ids=list(range(NCORES)))
    out = _unscramble([r["out"] for r in res.results])
    return out.reshape(B, 1024, 1, 1, 1)

